# revision 21
# baseline (speedup 1.0000x reference)
"""Trainium2 Bass kernel for BinarizedConvNet (6 binarized convs + BN + pool + 3 FC).

Data-parallel over batch (N=256 -> 32 images/core on 8 NeuronCores). Training-mode
BatchNorm needs full-batch stats: per-(layer, co-tile) channel stats are AllReduced
([128,3] f32); the per-co-tile split lets all but the last collective of a layer
overlap that layer's remaining matmuls. A dummy collective at t=0 absorbs the
cross-core launch skew so the first real one sees aligned peers.

Schedule notes (v3):
- conv1: 4x row-tiled K=32 matmuls (tile_position) over host-built im2col strips.
  psum->SBUF casts split 3:1 scalar:vector, per-slot f16 bn_stats, so conv1 is
  not vector-throughput-bound.
- max-pool runs BEFORE BN+relu (exact: pool commutes with monotone per-channel
  scale>0 affine + relu; scale = g/sigma with g=1>0): a single tensor_reduce(XY)
  per psum tile during the matmul phase. The post-collective apply is then a
  single fused scalar ACT over pooled data - 4x smaller and with no Vector ops
  that could head-of-line block the next layer's psum evacuation.
- all weights host-binarized (+-1 fp16); w2..w4 preloaded, w5/w6 streamed
  per-co-tile (2-deep ring); fc1's 16MB streamed: 29 chunks pre-staged into dead
  arena space, 8-slot window for the rest; fc2 prefetched into dead y5 space.
- psum: single 8-bank ring. Activations in flat arena A, conv outputs (raw or
  pooled) in flat arena B, slice-granular deps give cross-layer pipelining.
"""

import sys

sys.path.insert(0, "/opt/trn_rl_repo")

import numpy as np

import concourse.bass as bass  # noqa: F401
import concourse.mybir as mybir
import concourse.tile as tile
from concourse import bacc
from concourse.bass_utils import run_bass_kernel_spmd
from concourse.masks import make_identity

N_CORES = 8
N_LOC = 32
EPS = 1e-5
f32 = mybir.dt.float32
f16 = mybir.dt.float16
AF = mybir.ActivationFunctionType
OP = mybir.AluOpType
AX = mybir.AxisListType
RG = [list(range(N_CORES))]

# (ci, co, H, W, pool) per conv layer
CONV_CFG = [
    (3, 128, 32, 32, False),
    (128, 128, 32, 32, True),
    (128, 256, 16, 16, False),
    (256, 256, 16, 16, True),
    (256, 512, 8, 8, False),
    (512, 512, 8, 8, True),
]

# A-arena element offsets (fp16 elems per partition; slot = 36992)
A_ELEMS = 36992
OFF_X = {2: 0, 3: 0, 4: 10368, 5: 0, 6: 6400, 7: 0}  # 7 = xfc
SZ_X = {2: 36992, 3: 10368, 4: 20736, 5: 6400, 6: 12800, 7: 2048}
OFF_FW1A = 19200   # 17 fc1 chunks at [19200:36608)
# B-arena element offsets (slot = 32768)
B_ELEMS = 32768
B_Y = {1: 0, 3: 8192, 5: 4096}      # raw conv outs (non-pool layers)
B_YP = {2: 0, 4: 0, 6: 0}           # pooled raw outs (pool layers)
OFF_FW2 = 4096     # fc2 weights [4096:12288), lands after apply-l5
OFF_FC1WIN = 12288  # 8 streaming slots [12288:20480)
OFF_FW1B = 20480   # 8 pre-staged fc1 chunks
OFF_FW1C = 28672   # 4 pre-staged fc1 chunks

N_PRE_A, N_PRE_B, N_PRE_C = 17, 8, 4
PRE_TOT = N_PRE_A + N_PRE_B + N_PRE_C  # 29


def build(debug=False):
    nc = bacc.Bacc("TRN2", target_bir_lowering=False, debug=False, num_devices=N_CORES)

    xcol_in = nc.dram_tensor("xcol", [128, 8 * 1156], f16, kind="ExternalInput")
    w1_in = nc.dram_tensor("w1", [128, 128], f16, kind="ExternalInput")
    w2_in = nc.dram_tensor("w2", [128, 9 * 128], f16, kind="ExternalInput")
    w3_in = nc.dram_tensor("w3", [128, 9 * 256], f16, kind="ExternalInput")
    w4_in = nc.dram_tensor("w4", [128, 2 * 9 * 256], f16, kind="ExternalInput")
    w5s_in = nc.dram_tensor("w5s", [4, 128, 2 * 9 * 128], f16, kind="ExternalInput")
    w6s_in = nc.dram_tensor("w6s", [4, 128, 4 * 9 * 128], f16, kind="ExternalInput")
    g_in, bt_in = [None], [None]
    for l in range(1, 7):
        co = CONV_CFG[l - 1][1]
        g_in.append(nc.dram_tensor(f"g{l}", [co], f32, kind="ExternalInput"))
        bt_in.append(nc.dram_tensor(f"bt{l}", [co], f32, kind="ExternalInput"))
    fw1s_in = nc.dram_tensor("fw1s", [64, 128, 1024], f16, kind="ExternalInput")
    fw2t_in = nc.dram_tensor("fw2t", [1024, 1024], f16, kind="ExternalInput")
    fw3t_in = nc.dram_tensor("fw3t", [1024, 10], f32, kind="ExternalInput")
    fb1_in = nc.dram_tensor("fb1", [1, 1024], f16, kind="ExternalInput")
    fb2_in = nc.dram_tensor("fb2", [1, 1024], f16, kind="ExternalInput")
    fb3_in = nc.dram_tensor("fb3", [1, 10], f32, kind="ExternalInput")
    out = nc.dram_tensor("out", [N_LOC, 10], f32, kind="ExternalOutput")

    cc_in, cc_out = {}, {}
    for key in [(0, 0)] + [
        (l, ct) for l in range(1, 7) for ct in range(CONV_CFG[l - 1][1] // 128)
    ]:
        l, ct = key
        cc_in[key] = nc.dram_tensor(f"cci{l}_{ct}", [128, 2], f32)
        cc_out[key] = nc.dram_tensor(f"cco{l}_{ct}", [128, 2], f32, addr_space="Shared")

    dbg = {}
    if debug:
        for l, (ci, co, H, W, pool) in enumerate(CONV_CFG, start=1):
            Ho, Wo = (H // 2, W // 2) if pool else (H, W)
            dbg[f"y{l}"] = nc.dram_tensor(
                f"dbg_y{l}", [co, N_LOC * Ho * Wo], f16, kind="ExternalOutput"
            )
        dbg["xfc"] = nc.dram_tensor(
            "dbg_xfc", [512, N_LOC * 16], f16, kind="ExternalOutput"
        )
        dbg["yfc1"] = nc.dram_tensor("dbg_yfc1", [N_LOC, 1024], f16, kind="ExternalOutput")
        dbg["yfc2"] = nc.dram_tensor("dbg_yfc2", [N_LOC, 1024], f32, kind="ExternalOutput")

    ins = dict(
        xcol=xcol_in, w1=w1_in, w2=w2_in, w3=w3_in, w4=w4_in, w5s=w5s_in,
        w6s=w6s_in, g=g_in, bt=bt_in, fw1s=fw1s_in, fw2t=fw2t_in, fw3t=fw3t_in,
        fb1=fb1_in, fb2=fb2_in, fb3=fb3_in, out=out, cc_in=cc_in, cc_out=cc_out,
    )
    with tile.TileContext(nc) as tc:
        _emit(nc, tc, ins, dbg)
    nc.compile()
    return nc


def _emit(nc, tc, ins, dbg):
    n = N_LOC

    psum = tc.alloc_tile_pool(name="psum", bufs=1, space="PSUM")
    Ap = tc.alloc_tile_pool(name="arena_a", bufs=1)
    Bp = tc.alloc_tile_pool(name="arena_b", bufs=1)
    Wp = tc.alloc_tile_pool(name="wpool", bufs=1)
    misc = tc.alloc_tile_pool(name="misc", bufs=1)

    def acc_tile(nm):
        return psum.tile([128, 512], f32, tag="acc", bufs=8, name=nm)

    A = Ap.tile([128, A_ELEMS], f16, tag="A")
    B = Bp.tile([128, B_ELEMS], f16, tag="B")
    At, Bt = A[:], B[:]

    # ---------------- sync collective: absorb launch skew off-critical-path ----
    sk = misc.tile([128, 2], f32, tag="sk")
    nc.vector.memset(sk[:], 0.0)
    nc.sync.dma_start(out=ins["cc_in"][(0, 0)][:], in_=sk[:])
    nc.gpsimd.collective_compute(
        "AllReduce", OP.add, replica_groups=RG,
        ins=[ins["cc_in"][(0, 0)][:]], outs=[ins["cc_out"][(0, 0)][:]],
    )

    # ---------------- static weight / param loads ----------------
    w1t = misc.tile([128, 128], f16, tag="w1t")
    nc.sync.dma_start(out=w1t[:], in_=ins["w1"][:])
    # per-image-column loads so slot 0's strips land in ~1us, not after 7us
    xcol_iv = ins["xcol"][:].rearrange("p (i q) -> p i q", q=1156)
    xcv_dma = At[:, 0:9248].rearrange("p (i q) -> p i q", q=1156)
    for i in range(8):
        nc.sync.dma_start(out=xcv_dma[:, i], in_=xcol_iv[:, i])
    w2t = Wp.tile([128, 9 * 128], f16, tag="w2")
    nc.sync.dma_start(out=w2t[:], in_=ins["w2"][:])
    w3t = Wp.tile([128, 9 * 256], f16, tag="w3")
    nc.sync.dma_start(out=w3t[:], in_=ins["w3"][:])
    w4t = Wp.tile([128, 2 * 9 * 256], f16, tag="w4")
    nc.sync.dma_start(out=w4t[:], in_=ins["w4"][:])

    gts, btts = {}, {}
    for l in range(1, 7):
        co_t = CONV_CFG[l - 1][1] // 128
        gt = misc.tile([128, co_t], f32, tag="gt", bufs=6, name=f"gt{l}")
        btt = misc.tile([128, co_t], f32, tag="btt", bufs=6, name=f"btt{l}")
        nc.sync.dma_start(out=gt[:], in_=ins["g"][l][:].rearrange("(t c) -> c t", c=128))
        nc.sync.dma_start(out=btt[:], in_=ins["bt"][l][:].rearrange("(t c) -> c t", c=128))
        gts[l], btts[l] = gt, btt

    # ---------------- shared machinery ----------------
    def stats_and_collective(l, ct, st6v):
        mv = misc.tile([128, 2], f32, tag="mv", bufs=4, name=f"mv{l}_{ct}")
        nc.vector.bn_aggr(mv[:], st6v)
        # pk = [mean/8, (var + mean^2 + EPS)/8]: AllReduce-add then yields
        # [mean_g, E[y^2]_g + EPS] directly - minimal post-collective chain.
        pk = misc.tile([128, 2], f32, tag="pk", bufs=4, name=f"pk{l}_{ct}")
        nc.vector.tensor_scalar_mul(pk[:, 0:1], mv[:, 0:1], 1.0 / N_CORES)
        nc.vector.tensor_tensor(pk[:, 1:2], mv[:, 0:1], mv[:, 0:1], OP.mult)
        nc.vector.tensor_tensor(pk[:, 1:2], pk[:, 1:2], mv[:, 1:2], OP.add)
        nc.vector.tensor_scalar(
            pk[:, 1:2], pk[:, 1:2], EPS, 1.0 / N_CORES, OP.add, OP.mult
        )
        nc.sync.dma_start(out=ins["cc_in"][(l, ct)][:], in_=pk[:])
        nc.gpsimd.collective_compute(
            "AllReduce", OP.add, replica_groups=RG,
            ins=[ins["cc_in"][(l, ct)][:]], outs=[ins["cc_out"][(l, ct)][:]],
        )
        gl = misc.tile([128, 2], f32, tag="gl", bufs=4, name=f"gl{l}_{ct}")
        nc.sync.dma_start(out=gl[:], in_=ins["cc_out"][(l, ct)][:])
        return gl

    def finalize(l, ct, gl):
        var = misc.tile([128, 1], f32, tag="var", bufs=4, name=f"var{l}_{ct}")
        msq = misc.tile([128, 1], f32, tag="msq", bufs=4, name=f"msq{l}_{ct}")
        inv = misc.tile([128, 1], f32, tag="inv", bufs=4, name=f"inv{l}_{ct}")
        sc = misc.tile([128, 1], f32, tag="sc", bufs=4, name=f"sc{l}_{ct}")
        bi = misc.tile([128, 1], f32, tag="bi", bufs=4, name=f"bi{l}_{ct}")
        nc.vector.tensor_tensor(msq[:], gl[:, 0:1], gl[:, 0:1], OP.mult)
        nc.vector.tensor_tensor(var[:], gl[:, 1:2], msq[:], OP.subtract)
        nc.scalar.activation(msq[:], var[:], AF.Sqrt)  # msq = std
        nc.vector.reciprocal(inv[:], msq[:])
        nc.vector.tensor_tensor(sc[:], gts[l][:, ct : ct + 1], inv[:], OP.mult)
        nc.vector.tensor_tensor(bi[:], gl[:, 0:1], sc[:], OP.mult)
        nc.vector.tensor_tensor(bi[:], btts[l][:, ct : ct + 1], bi[:], OP.subtract)
        return sc, bi

    def emit_apply(srcv, nv_int, Ha, Wa, sc, bi):
        """relu(sc*src + bi) per image chunk. srcv: [128, n, Ha, Wa] (raw or
        pooled); nv_int: dst interior sliceable by image."""
        ich = min(n, max(1, 1024 // (Ha * Wa)))
        for ch in range(n // ich):
            i0, i1 = ch * ich, (ch + 1) * ich
            nc.scalar.activation(
                nv_int[:, i0:i1], srcv[:, i0:i1], AF.Relu, bias=bi[:], scale=sc[:]
            )

    def memset_borders(xv_full, Hn, Wn):
        nc.gpsimd.memset(xv_full[:, :, 0 : Hn : Hn - 1, :], 0.0)
        nc.gpsimd.memset(xv_full[:, :, 1 : Hn - 1, 0 : Wn : Wn - 1], 0.0)

    # ---------------- layer 1: row-tiled K=32 im2col conv ----------------
    xcv = At[:, 0:9248].rearrange("p (i h w) -> p i h w", h=34, w=34)
    y1 = Bt[:, B_Y[1] : B_Y[1] + 32768]
    y1v = y1.rearrange("p (i hf q) -> p i hf q", hf=2, q=512)
    st6l1 = misc.tile([128, 64 * 6], f32, tag="st6l1", name="st6_l1")
    st6v1 = st6l1[:].rearrange("p (t s) -> p t s", s=6)
    for s in range(16):
        i, hf = s // 2, s % 2
        hh = hf * 16
        accs = []
        for k in range(4):
            a = acc_tile(f"l1a{s}_{k}")
            nc.tensor.matmul(
                a[:], w1t[32 * k : 32 * k + 32, :],
                xcv[32 * k : 32 * k + 32, i, hh + 1 : hh + 17, 1:33],
                start=True, stop=True, tile_position=(32 * k, 0),
            )
            accs.append(a)
        for k in range(4):
            # stats straight from psum (vector) so the collective isn't gated
            # on the scalar casts; casts may lag into the collective window.
            nc.vector.bn_stats(st6v1[:, s * 4 + k], accs[k][:])
            nc.scalar.copy(y1v[:, 8 * k + i, hf], accs[k][:])

    x2v = At[:, 0:36992].rearrange("p (i h w) -> p i h w", h=34, w=34)
    memset_borders(x2v, 34, 34)

    gl1 = stats_and_collective(1, 0, st6v1)
    sc1, bi1 = finalize(1, 0, gl1)
    emit_apply(
        y1.rearrange("p (i h w) -> p i h w", h=32, w=32),
        x2v[:, :, 1:33, 1:33], 32, 32, sc1, bi1,
    )
    if "y1" in dbg:
        nc.sync.dma_start(out=dbg["y1"][:], in_=y1)

    # ---------------- conv layers 2..6 ----------------
    def conv_layer(l):
        ci, co, H, W, do_pool = CONV_CFG[l - 1]
        ci_t, co_t = max(1, ci // 128), co // 128
        npix = n * H * W
        ntile = npix // 512
        ipt = 512 // (H * W) if H * W <= 512 else 0
        Ho, Wo = (H // 2, W // 2) if do_pool else (H, W)
        npo = n * Ho * Wo

        if do_pool:
            yp = Bt[:, B_YP[l] : B_YP[l] + co_t * npo]
            yr = None
        else:
            yr = Bt[:, B_Y[l] : B_Y[l] + co_t * npix]
            yp = None

        if l == 2:
            srcv = x2v
        else:
            o0 = OFF_X[l]
            srcv = At[:, o0 : o0 + SZ_X[l]].rearrange(
                "p (t i h w) -> p t i h w", t=ci_t, h=H + 2, w=W + 2
            )
        o1 = OFF_X[l + 1]
        if l < 6:
            Hn, Wn = Ho + 2, Wo + 2
            nxtv = At[:, o1 : o1 + SZ_X[l + 1]].rearrange(
                "p (t i h w) -> p t i h w", t=co_t, h=Hn, w=Wn
            )
        else:
            nxtv = At[:, o1 : o1 + SZ_X[7]].rearrange(
                "p (t i q) -> p t i q", t=co_t, q=16
            )

        # padded-output borders: for l>=3 the dst region only overlaps data
        # dead by this layer's start, so zero it up front (off critical path);
        # l2's dst lives inside x2 (this layer's source) - done after tiles.
        if l in (3, 4, 5):
            nxtf = At[:, o1 : o1 + SZ_X[l + 1]].rearrange(
                "p (a h w) -> p a h w", h=Ho + 2, w=Wo + 2
            )
            memset_borders(nxtf, Ho + 2, Wo + 2)

        if l == 2:
            wv = w2t[:].rearrange("p (o c) -> p o c", o=9)
            wget = lambda t, o, ct: wv[:, o, :]
        elif l == 3:
            wv = w3t[:].rearrange("p (o c) -> p o c", o=9)
            wget = lambda t, o, ct: wv[:, o, ct * 128 : (ct + 1) * 128]
        elif l == 4:
            wv = w4t[:].rearrange("p (t o c) -> p t o c", t=2, o=9)
            wget = lambda t, o, ct: wv[:, t, o, ct * 128 : (ct + 1) * 128]
        else:
            wget = None

        for ct in range(co_t):
            if l in (5, 6):
                wc = Wp.tile(
                    [128, ci_t * 9 * 128], f16, tag=f"w{l}s", bufs=2, name=f"w{l}c{ct}"
                )
                nc.sync.dma_start(out=wc[:], in_=ins[f"w{l}s"][ct])
                wcv = wc[:].rearrange("p (t o c) -> p t o c", t=ci_t, o=9)
                wget = lambda t, o, _ct, wcv=wcv: wcv[:, t, o, :]
            st6 = misc.tile([128, 64 * 6], f32, tag="st6", bufs=2, name=f"st6_{l}_{ct}")
            st6v = st6[:, 0 : ntile * 6].rearrange("p (t s) -> p t s", s=6)
            for pt in range(ntile):
                a = acc_tile(f"a{l}_{ct}_{pt}")
                first = True
                for t in range(ci_t):
                    for dh in range(3):
                        for dw in range(3):
                            o = dh * 3 + dw
                            if ipt == 0:
                                img, hh = pt // 2, (pt % 2) * 16
                                rhs = srcv[:, img, hh + dh : hh + dh + 16, dw : dw + 32]
                            else:
                                i0 = pt * ipt
                                rhs = srcv[:, t, i0 : i0 + ipt, dh : dh + H, dw : dw + W]
                            nc.tensor.matmul(
                                a[:], wget(t, o, ct), rhs,
                                start=first, stop=(t == ci_t - 1 and o == 8),
                            )
                            first = False
                if not do_pool:
                    ydst = yr[:, ct * npix + pt * 512 : ct * npix + (pt + 1) * 512]
                    nc.scalar.copy(ydst, a[:])
                    nc.vector.bn_stats(st6v[:, pt], ydst)
                else:
                    nc.vector.bn_stats(st6v[:, pt], a[:])
                if do_pool:
                    # 2x2 max-pool straight from psum (commutes with the later
                    # monotone scale>0 BN+relu)
                    if ipt == 0:  # l2: half-image tile, 16x32 px
                        img, hf = pt // 2, pt % 2
                        av = a[:].rearrange(
                            "p (hp qh wp qw) -> p hp wp qh qw", hp=8, qh=2, qw=2
                        )
                        ypv2 = yp.rearrange("p (i h w) -> p i h w", h=16, w=16)
                        nc.vector.tensor_reduce(
                            ypv2[:, img, hf * 8 : hf * 8 + 8, :], av, AX.XY, OP.max
                        )
                    else:
                        av = a[:].rearrange(
                            "p (i hp qh wp qw) -> p i hp wp qh qw",
                            i=ipt, hp=H // 2, qh=2, qw=2,
                        )
                        ypv = yp.rearrange(
                            "p (t i h w) -> p t i h w", t=co_t, h=Ho, w=Wo
                        )
                        i0 = pt * ipt
                        nc.vector.tensor_reduce(
                            ypv[:, ct, i0 : i0 + ipt], av, AX.XY, OP.max
                        )
                else:
                    pass  # handled above (scalar copy + f16 stats)

            if l == 2:  # x3 borders: right after l2's matmuls, before apply
                nxtf = At[:, o1 : o1 + SZ_X[3]].rearrange(
                    "p (a h w) -> p a h w", h=Ho + 2, w=Wo + 2
                )
                memset_borders(nxtf, Ho + 2, Wo + 2)

            gl = stats_and_collective(l, ct, st6v)
            sc, bi = finalize(l, ct, gl)
            if l < 6:
                dst = nxtv[:, ct, :, 1 : Ho + 1, 1 : Wo + 1]
            else:
                dst = nxtv[:, ct]
            if do_pool:
                src_ap = yp.rearrange(
                    "p (t i h w) -> p t i h w", t=co_t, h=Ho, w=Wo
                )[:, ct]
            else:
                src_ap = yr[:, ct * npix : (ct + 1) * npix].rearrange(
                    "p (i h w) -> p i h w", h=H, w=W
                )
            emit_apply(src_ap, dst, Ho, Wo, sc, bi)

        # fc-weight pre-staging in dead arena space
        if l == 4:
            for m in range(N_PRE_A, N_PRE_A + N_PRE_B):
                o = OFF_FW1B + (m - N_PRE_A) * 1024
                nc.sync.dma_start(out=Bt[:, o : o + 1024], in_=ins["fw1s"][m])
            for m in range(N_PRE_A + N_PRE_B, PRE_TOT):
                o = OFF_FW1C + (m - N_PRE_A - N_PRE_B) * 1024
                nc.sync.dma_start(out=Bt[:, o : o + 1024], in_=ins["fw1s"][m])
        if l == 5:
            for m in range(N_PRE_A):
                o = OFF_FW1A + m * 1024
                nc.sync.dma_start(out=At[:, o : o + 1024], in_=ins["fw1s"][m])
        if l == 6:
            for jt in range(8):
                o = OFF_FW2 + jt * 1024
                nc.sync.dma_start(
                    out=Bt[:, o : o + 1024],
                    in_=ins["fw2t"][jt * 128 : (jt + 1) * 128, :],
                )

        if f"y{l}" in dbg:
            src = yp if do_pool else yr
            sz = npo if do_pool else npix
            for ct in range(co_t):
                nc.sync.dma_start(
                    out=dbg[f"y{l}"][ct * 128 : (ct + 1) * 128, :],
                    in_=src[:, ct * sz : (ct + 1) * sz],
                )

    for l in range(2, 7):
        conv_layer(l)

    # ---------------- FC ----------------
    xfcv = At[:, 0:2048].rearrange("p (t i q) -> p t i q", t=4, q=16)
    if "xfc" in dbg:
        for t in range(4):
            nc.sync.dma_start(
                out=dbg["xfc"][t * 128 : (t + 1) * 128, :], in_=xfcv[:, t]
            )

    fb1b = misc.tile([1, 1024], f16, tag="fb1b")
    nc.sync.dma_start(out=fb1b[:], in_=ins["fb1"][:])
    fb2b = misc.tile([1, 1024], f16, tag="fb2b")
    nc.sync.dma_start(out=fb2b[:], in_=ins["fb2"][:])
    fb3f = misc.tile([1, 10], f32, tag="fb3f")
    nc.sync.dma_start(out=fb3f[:], in_=ins["fb3"][:])
    ones_b = misc.tile([1, n], f16, tag="ones_b")
    nc.vector.memset(ones_b[:], 1.0)
    ones_f = misc.tile([1, n], f32, tag="ones_f")
    nc.vector.memset(ones_f[:], 1.0)
    idb = misc.tile([n, n], f16, tag="idb")
    make_identity(nc, idb[:])
    idf = misc.tile([n, n], f32, tag="idf")
    make_identity(nc, idf[:])

    acc_h = [acc_tile(f"fc1acc{h}") for h in range(2)]
    fw1pa = At[:, OFF_FW1A : OFF_FW1A + N_PRE_A * 1024].rearrange(
        "p (m q) -> p m q", q=1024
    )
    for m in range(64):
        ct, p = divmod(m, 16)
        if m < N_PRE_A:
            wch = fw1pa[:, m]
        elif m < N_PRE_A + N_PRE_B:
            o = OFF_FW1B + (m - N_PRE_A) * 1024
            wch = Bt[:, o : o + 1024]
        elif m < PRE_TOT:
            o = OFF_FW1C + (m - N_PRE_A - N_PRE_B) * 1024
            wch = Bt[:, o : o + 1024]
        else:
            slot = (m - PRE_TOT) % 14
            if slot < 8:
                wch = Bt[:, OFF_FC1WIN + slot * 1024 : OFF_FC1WIN + (slot + 1) * 1024]
            elif slot < 12:  # dead x5 space in arena A
                o = 2048 + (slot - 8) * 1024
                wch = At[:, o : o + 1024]
            else:  # dead yp4 space in arena B
                o = 2048 + (slot - 12) * 1024
                wch = Bt[:, o : o + 1024]
            nc.sync.dma_start(out=wch, in_=ins["fw1s"][m])
        lhsT = xfcv[:, ct, :, p]
        for hh in range(2):
            nc.tensor.matmul(
                acc_h[hh][0:32, :], lhsT, wch[:, hh * 512 : (hh + 1) * 512],
                start=(m == 0), stop=False,
            )
    y1fc = misc.tile([n, 1024], f16, tag="y1fc")
    for hh in range(2):
        nc.tensor.matmul(
            acc_h[hh][0:32, :], ones_b[:], fb1b[:, hh * 512 : (hh + 1) * 512],
            start=False, stop=True,
        )
        nc.scalar.activation(
            y1fc[:, hh * 512 : (hh + 1) * 512], acc_h[hh][0:32, :], AF.Relu
        )
    if "yfc1" in dbg:
        nc.sync.dma_start(out=dbg["yfc1"][:], in_=y1fc[:])

    y1t = misc.tile([128, 8 * n], f16, tag="y1t")
    y1tv = y1t[:].rearrange("p (t i) -> p t i", t=8)
    tps = []
    for jt in range(8):
        tp = acc_tile(f"tr1_{jt}")
        tpb = tp[:].bitcast(f16)[:, 0:n]
        nc.tensor.transpose(tpb, y1fc[:, jt * 128 : (jt + 1) * 128], idb[:])
        tps.append(tpb)
    for jt in range(8):
        nc.vector.tensor_copy(y1tv[:, jt], tps[jt])

    w2fv = Bt[:, OFF_FW2 : OFF_FW2 + 8 * 1024].rearrange("p (t q) -> p t q", q=1024)
    y2fc = misc.tile([n, 1024], f32, tag="y2fc")
    for hh in range(2):
        a2 = acc_tile(f"fc2acc{hh}")
        for jt in range(8):
            nc.tensor.matmul(
                a2[0:32, :], y1tv[:, jt], w2fv[:, jt, hh * 512 : (hh + 1) * 512],
                start=(jt == 0), stop=False,
            )
        nc.tensor.matmul(
            a2[0:32, :], ones_b[:], fb2b[:, hh * 512 : (hh + 1) * 512],
            start=False, stop=True,
        )
        nc.scalar.activation(
            y2fc[:, hh * 512 : (hh + 1) * 512], a2[0:32, :], AF.Relu
        )
    if "yfc2" in dbg:
        nc.sync.dma_start(out=dbg["yfc2"][:], in_=y2fc[:])

    y2t = misc.tile([128, 8 * n], f32, tag="y2t")
    y2tv = y2t[:].rearrange("p (t i) -> p t i", t=8)
    tps2 = []
    for it in range(8):
        tp = acc_tile(f"tr2_{it}")
        tpf = tp[:][:, 0:n]
        nc.tensor.transpose(tpf, y2fc[:, it * 128 : (it + 1) * 128], idf[:])
        tps2.append(tpf)
    for it in range(8):
        nc.vector.tensor_copy(y2tv[:, it], tps2[it])
    w3fc = misc.tile([128, 8 * 10], f32, tag="w3fc")
    w3v = w3fc[:].rearrange("p (t j) -> p t j", j=10)
    nc.sync.dma_start(out=w3v, in_=ins["fw3t"][:].rearrange("(t c) j -> c t j", c=128))
    a3 = acc_tile("fc3acc")
    for it in range(8):
        nc.tensor.matmul(
            a3[0:n, 0:10], y2tv[:, it], w3v[:, it], start=(it == 0), stop=False
        )
    nc.tensor.matmul(a3[0:n, 0:10], ones_f[:], fb3f[:], start=False, stop=True)
    out_sb = misc.tile([n, 10], f32, tag="out_sb")
    nc.scalar.copy(out_sb[:], a3[0:n, 0:10])
    nc.sync.dma_start(out=ins["out"][:], in_=out_sb[:])

    for p in (misc, Wp, Bp, Ap, psum):
        p.release()


# ---------------------------------------------------------------------------
# host-side wrapper (layout/transpose/binarize only)
# ---------------------------------------------------------------------------

_CACHE = {}


def _binarize(a):
    return np.where(np.asarray(a, np.float32) >= 0, 1.0, -1.0).astype(np.float32)


def _prep_inputs(inputs):
    h = np.float16
    sh = {}
    w1b = _binarize(inputs["cw1"])  # [128, 3, 3, 3] OIHW
    w1c = w1b.transpose(2, 3, 1, 0).reshape(27, 128)
    w1col = np.zeros((128, 128), np.float32)
    for k in range(4):
        w1col[32 * k : 32 * k + 27] = w1c
    sh["w1"] = w1col.astype(h)

    def conv_w(l):
        cw = _binarize(inputs[f"cw{l}"])  # [co, ci, 3, 3]
        co, ci = cw.shape[0], cw.shape[1]
        arr = cw.transpose(2, 3, 1, 0).reshape(9, ci, co)  # [o, ci, co]
        t = ci // 128
        a = arr.transpose(1, 0, 2).reshape(t, 128, 9, co).transpose(1, 0, 2, 3)
        return np.ascontiguousarray(a.reshape(128, t * 9 * co)).astype(h)

    sh["w2"], sh["w3"], sh["w4"] = conv_w(2), conv_w(3), conv_w(4)

    def conv_w_ct(l):
        cw = _binarize(inputs[f"cw{l}"])
        co, ci = cw.shape[0], cw.shape[1]
        arr = cw.transpose(2, 3, 1, 0).reshape(9, ci, co)
        t, nct = ci // 128, co // 128
        outw = np.zeros((nct, 128, t * 9 * 128), np.float32)
        for c in range(nct):
            ch = arr[:, :, c * 128 : (c + 1) * 128]
            a = ch.transpose(1, 0, 2).reshape(t, 128, 9, 128).transpose(1, 0, 2, 3)
            outw[c] = a.reshape(128, t * 9 * 128)
        return outw.astype(h)

    sh["w5s"], sh["w6s"] = conv_w_ct(5), conv_w_ct(6)
    for l in range(1, 7):
        sh[f"g{l}"] = np.ascontiguousarray(inputs[f"g{l}"], np.float32)
        sh[f"bt{l}"] = np.ascontiguousarray(inputs[f"bt{l}"], np.float32)
    fw1 = _binarize(inputs["fw1"])  # [1024, 8192]
    a = fw1.reshape(1024, 512, 16).transpose(1, 2, 0)  # [ch, px, out]
    a = a.reshape(4, 128, 16, 1024).transpose(0, 2, 1, 3)  # [ct, px, part, out]
    sh["fw1s"] = np.ascontiguousarray(a.reshape(64, 128, 1024)).astype(h)
    sh["fw2t"] = np.ascontiguousarray(_binarize(inputs["fw2"]).T).astype(h)
    sh["fw3t"] = np.ascontiguousarray(np.asarray(inputs["fw3"], np.float32).T)
    sh["fb1"] = np.asarray(inputs["fb1"], np.float32).reshape(1, 1024).astype(h)
    sh["fb2"] = np.asarray(inputs["fb2"], np.float32).reshape(1, 1024).astype(h)
    sh["fb3"] = np.ascontiguousarray(
        np.asarray(inputs["fb3"], np.float32).reshape(1, 10)
    )

    x = np.asarray(inputs["x"], np.float32)
    xp = np.zeros((256, 3, 34, 34), np.float32)
    xp[:, :, 1:33, 1:33] = x
    xpf = xp.reshape(256, 3, 1156)
    taps = [(dh, dw) for dh in range(3) for dw in range(3)]
    in_maps = []
    for c in range(N_CORES):
        xc = np.zeros((128, 8, 1156), np.float32)
        for k in range(4):
            imgs = xpf[c * 32 + 8 * k : c * 32 + 8 * k + 8]
            for o, (dh, dw) in enumerate(taps):
                s = (dh - 1) * 34 + (dw - 1)
                d0, d1 = max(0, -s), 1156 - max(0, s)
                xc[32 * k + o * 3 : 32 * k + o * 3 + 3, :, d0:d1] = imgs[
                    :, :, d0 + s : d1 + s
                ].transpose(1, 0, 2)
        m = dict(sh)
        m["xcol"] = xc.reshape(128, 8 * 1156).astype(h)
        in_maps.append(m)
    return in_maps


def run(inputs, debug=False, trace=False):
    key = "dbg" if debug else "rel"
    if key not in _CACHE:
        _CACHE[key] = build(debug=debug)
    nc = _CACHE[key]
    in_maps = _prep_inputs(inputs)
    res = run_bass_kernel_spmd(nc, in_maps, core_ids=list(range(N_CORES)), trace=trace)
    outs = np.concatenate([r["out"] for r in res.results], axis=0)
    return outs, res


def kernel(**inputs) -> np.ndarray:
    outs, _ = run(inputs, debug=False, trace=False)
    return outs


# revision 22
# speedup vs baseline: 1.0335x; 1.0335x over previous
"""Trainium2 Bass kernel for BinarizedConvNet (6 binarized convs + BN + pool + 3 FC).

Data-parallel over batch (N=256 -> 32 images/core on 8 NeuronCores). Training-mode
BatchNorm needs full-batch stats: per-(layer, co-tile) channel stats are AllReduced
([128,3] f32); the per-co-tile split lets all but the last collective of a layer
overlap that layer's remaining matmuls. A dummy collective at t=0 absorbs the
cross-core launch skew so the first real one sees aligned peers.

Schedule notes (v3):
- conv1: 4x row-tiled K=32 matmuls (tile_position) over host-built im2col strips.
  psum->SBUF casts split 3:1 scalar:vector, per-slot f16 bn_stats, so conv1 is
  not vector-throughput-bound.
- max-pool runs BEFORE BN+relu (exact: pool commutes with monotone per-channel
  scale>0 affine + relu; scale = g/sigma with g=1>0): a single tensor_reduce(XY)
  per psum tile during the matmul phase. The post-collective apply is then a
  single fused scalar ACT over pooled data - 4x smaller and with no Vector ops
  that could head-of-line block the next layer's psum evacuation.
- all weights host-binarized (+-1 fp16); w2..w4 preloaded, w5/w6 streamed
  per-co-tile (2-deep ring); fc1's 16MB streamed: 29 chunks pre-staged into dead
  arena space, 8-slot window for the rest; fc2 prefetched into dead y5 space.
- psum: single 8-bank ring. Activations in flat arena A, conv outputs (raw or
  pooled) in flat arena B, slice-granular deps give cross-layer pipelining.
"""

import sys

sys.path.insert(0, "/opt/trn_rl_repo")

import numpy as np

import concourse.bass as bass  # noqa: F401
import concourse.mybir as mybir
import concourse.tile as tile
from concourse import bacc
from concourse.bass_utils import run_bass_kernel_spmd
from concourse.masks import make_identity

N_CORES = 8
N_LOC = 32
EPS = 1e-5
f32 = mybir.dt.float32
f16 = mybir.dt.float16
AF = mybir.ActivationFunctionType
OP = mybir.AluOpType
AX = mybir.AxisListType
RG = [list(range(N_CORES))]

# (ci, co, H, W, pool) per conv layer
CONV_CFG = [
    (3, 128, 32, 32, False),
    (128, 128, 32, 32, True),
    (128, 256, 16, 16, False),
    (256, 256, 16, 16, True),
    (256, 512, 8, 8, False),
    (512, 512, 8, 8, True),
]

# A-arena element offsets (fp16 elems per partition; slot = 36992)
A_ELEMS = 36992
OFF_X = {2: 0, 3: 0, 4: 10368, 5: 0, 6: 6400, 7: 0}  # 7 = xfc
SZ_X = {2: 36992, 3: 10368, 4: 20736, 5: 6400, 6: 12800, 7: 2048}
OFF_FW1A = 19200   # 17 fc1 chunks at [19200:36608)
# B-arena element offsets (slot = 32768)
B_ELEMS = 32768
B_Y = {1: 0, 3: 8192, 5: 4096}      # raw conv outs (non-pool layers)
B_YP = {2: 0, 4: 0, 6: 0}           # pooled raw outs (pool layers)
OFF_FW2 = 4096     # fc2 weights [4096:12288), lands after apply-l5
OFF_FC1WIN = 12288  # 8 streaming slots [12288:20480)
OFF_FW1B = 20480   # 8 pre-staged fc1 chunks
OFF_FW1C = 28672   # 4 pre-staged fc1 chunks

N_PRE_A, N_PRE_B, N_PRE_C = 17, 8, 4
PRE_TOT = N_PRE_A + N_PRE_B + N_PRE_C  # 29


def build(debug=False):
    nc = bacc.Bacc("TRN2", target_bir_lowering=False, debug=False, num_devices=N_CORES)

    xcol_in = nc.dram_tensor("xcol", [128, 8 * 1156], f16, kind="ExternalInput")
    w1_in = nc.dram_tensor("w1", [128, 128], f16, kind="ExternalInput")
    w2_in = nc.dram_tensor("w2", [128, 9 * 128], f16, kind="ExternalInput")
    w3_in = nc.dram_tensor("w3", [128, 9 * 256], f16, kind="ExternalInput")
    w4_in = nc.dram_tensor("w4", [128, 2 * 9 * 256], f16, kind="ExternalInput")
    w5s_in = nc.dram_tensor("w5s", [4, 128, 2 * 9 * 128], f16, kind="ExternalInput")
    w6s_in = nc.dram_tensor("w6s", [4, 128, 4 * 9 * 128], f16, kind="ExternalInput")
    g_in, bt_in = [None], [None]
    for l in range(1, 7):
        co = CONV_CFG[l - 1][1]
        g_in.append(nc.dram_tensor(f"g{l}", [co], f32, kind="ExternalInput"))
        bt_in.append(nc.dram_tensor(f"bt{l}", [co], f32, kind="ExternalInput"))
    fw1s_in = nc.dram_tensor("fw1s", [64, 128, 1024], f16, kind="ExternalInput")
    fw2t_in = nc.dram_tensor("fw2t", [1024, 1024], f16, kind="ExternalInput")
    fw3t_in = nc.dram_tensor("fw3t", [1024, 10], f32, kind="ExternalInput")
    fb1_in = nc.dram_tensor("fb1", [1, 1024], f16, kind="ExternalInput")
    fb2_in = nc.dram_tensor("fb2", [1, 1024], f16, kind="ExternalInput")
    fb3_in = nc.dram_tensor("fb3", [1, 10], f32, kind="ExternalInput")
    out = nc.dram_tensor("out", [N_LOC, 10], f32, kind="ExternalOutput")

    cc_in, cc_out = {}, {}
    for key in [(0, 0)] + [
        (l, ct) for l in range(1, 7) for ct in range(CONV_CFG[l - 1][1] // 128)
    ]:
        l, ct = key
        cc_in[key] = nc.dram_tensor(f"cci{l}_{ct}", [128, 2], f32)
        cc_out[key] = nc.dram_tensor(f"cco{l}_{ct}", [128, 2], f32, addr_space="Shared")

    dbg = {}
    if debug:
        for l, (ci, co, H, W, pool) in enumerate(CONV_CFG, start=1):
            Ho, Wo = (H // 2, W // 2) if pool else (H, W)
            dbg[f"y{l}"] = nc.dram_tensor(
                f"dbg_y{l}", [co, N_LOC * Ho * Wo], f16, kind="ExternalOutput"
            )
        dbg["xfc"] = nc.dram_tensor(
            "dbg_xfc", [512, N_LOC * 16], f16, kind="ExternalOutput"
        )
        dbg["yfc1"] = nc.dram_tensor("dbg_yfc1", [N_LOC, 1024], f16, kind="ExternalOutput")
        dbg["yfc2"] = nc.dram_tensor("dbg_yfc2", [N_LOC, 1024], f32, kind="ExternalOutput")

    ins = dict(
        xcol=xcol_in, w1=w1_in, w2=w2_in, w3=w3_in, w4=w4_in, w5s=w5s_in,
        w6s=w6s_in, g=g_in, bt=bt_in, fw1s=fw1s_in, fw2t=fw2t_in, fw3t=fw3t_in,
        fb1=fb1_in, fb2=fb2_in, fb3=fb3_in, out=out, cc_in=cc_in, cc_out=cc_out,
    )
    with tile.TileContext(nc) as tc:
        _emit(nc, tc, ins, dbg)
    nc.compile()
    return nc


def _emit(nc, tc, ins, dbg):
    n = N_LOC

    psum = tc.alloc_tile_pool(name="psum", bufs=1, space="PSUM")
    Ap = tc.alloc_tile_pool(name="arena_a", bufs=1)
    Bp = tc.alloc_tile_pool(name="arena_b", bufs=1)
    Wp = tc.alloc_tile_pool(name="wpool", bufs=1)
    misc = tc.alloc_tile_pool(name="misc", bufs=1)

    def acc_tile(nm):
        return psum.tile([128, 512], f32, tag="acc", bufs=8, name=nm)

    A = Ap.tile([128, A_ELEMS], f16, tag="A")
    B = Bp.tile([128, B_ELEMS], f16, tag="B")
    At, Bt = A[:], B[:]

    # ---------------- sync collective: absorb launch skew off-critical-path ----
    sk = misc.tile([128, 2], f32, tag="sk")
    nc.vector.memset(sk[:], 0.0)
    nc.sync.dma_start(out=ins["cc_in"][(0, 0)][:], in_=sk[:])
    nc.gpsimd.collective_compute(
        "AllReduce", OP.add, replica_groups=RG,
        ins=[ins["cc_in"][(0, 0)][:]], outs=[ins["cc_out"][(0, 0)][:]],
    )

    # ---------------- static weight / param loads ----------------
    w1t = misc.tile([128, 128], f16, tag="w1t")
    nc.sync.dma_start(out=w1t[:], in_=ins["w1"][:])
    # per-image-column loads so slot 0's strips land in ~1us, not after 7us
    xcol_iv = ins["xcol"][:].rearrange("p (i q) -> p i q", q=1156)
    xcv_dma = At[:, 0:9248].rearrange("p (i q) -> p i q", q=1156)
    for i in range(8):
        nc.sync.dma_start(out=xcv_dma[:, i], in_=xcol_iv[:, i])
    w2t = Wp.tile([128, 9 * 128], f16, tag="w2")
    nc.sync.dma_start(out=w2t[:], in_=ins["w2"][:])
    w3t = Wp.tile([128, 9 * 256], f16, tag="w3")
    nc.sync.dma_start(out=w3t[:], in_=ins["w3"][:])
    w4t = Wp.tile([128, 2 * 9 * 256], f16, tag="w4")
    nc.sync.dma_start(out=w4t[:], in_=ins["w4"][:])

    gts, btts = {}, {}
    for l in range(1, 7):
        co_t = CONV_CFG[l - 1][1] // 128
        gt = misc.tile([128, co_t], f32, tag="gt", bufs=6, name=f"gt{l}")
        btt = misc.tile([128, co_t], f32, tag="btt", bufs=6, name=f"btt{l}")
        nc.sync.dma_start(out=gt[:], in_=ins["g"][l][:].rearrange("(t c) -> c t", c=128))
        nc.sync.dma_start(out=btt[:], in_=ins["bt"][l][:].rearrange("(t c) -> c t", c=128))
        gts[l], btts[l] = gt, btt

    # ---------------- shared machinery ----------------
    def stats_and_collective(l, ct, st6v):
        mv = misc.tile([128, 2], f32, tag="mv", bufs=4, name=f"mv{l}_{ct}")
        nc.vector.bn_aggr(mv[:], st6v)
        # pk = [mean/8, (var + mean^2 + EPS)/8]: AllReduce-add then yields
        # [mean_g, E[y^2]_g + EPS] directly - minimal post-collective chain.
        pk = misc.tile([128, 2], f32, tag="pk", bufs=4, name=f"pk{l}_{ct}")
        nc.vector.tensor_scalar_mul(pk[:, 0:1], mv[:, 0:1], 1.0 / N_CORES)
        nc.vector.tensor_tensor(pk[:, 1:2], mv[:, 0:1], mv[:, 0:1], OP.mult)
        nc.vector.tensor_tensor(pk[:, 1:2], pk[:, 1:2], mv[:, 1:2], OP.add)
        nc.vector.tensor_scalar(
            pk[:, 1:2], pk[:, 1:2], EPS, 1.0 / N_CORES, OP.add, OP.mult
        )
        nc.sync.dma_start(out=ins["cc_in"][(l, ct)][:], in_=pk[:])
        nc.gpsimd.collective_compute(
            "AllReduce", OP.add, replica_groups=RG,
            ins=[ins["cc_in"][(l, ct)][:]], outs=[ins["cc_out"][(l, ct)][:]],
        )
        gl = misc.tile([128, 2], f32, tag="gl", bufs=4, name=f"gl{l}_{ct}")
        nc.sync.dma_start(out=gl[:], in_=ins["cc_out"][(l, ct)][:])
        return gl

    def finalize(l, ct, gl):
        var = misc.tile([128, 1], f32, tag="var", bufs=4, name=f"var{l}_{ct}")
        msq = misc.tile([128, 1], f32, tag="msq", bufs=4, name=f"msq{l}_{ct}")
        inv = misc.tile([128, 1], f32, tag="inv", bufs=4, name=f"inv{l}_{ct}")
        sc = misc.tile([128, 1], f32, tag="sc", bufs=4, name=f"sc{l}_{ct}")
        bi = misc.tile([128, 1], f32, tag="bi", bufs=4, name=f"bi{l}_{ct}")
        nc.vector.tensor_tensor(msq[:], gl[:, 0:1], gl[:, 0:1], OP.mult)
        nc.vector.tensor_tensor(var[:], gl[:, 1:2], msq[:], OP.subtract)
        nc.scalar.activation(msq[:], var[:], AF.Sqrt)  # msq = std
        nc.vector.reciprocal(inv[:], msq[:])
        nc.vector.tensor_tensor(sc[:], gts[l][:, ct : ct + 1], inv[:], OP.mult)
        nc.vector.tensor_tensor(bi[:], gl[:, 0:1], sc[:], OP.mult)
        nc.vector.tensor_tensor(bi[:], btts[l][:, ct : ct + 1], bi[:], OP.subtract)
        return sc, bi

    def emit_apply(srcv, nv_int, Ha, Wa, sc, bi):
        """relu(sc*src + bi) per image chunk. srcv: [128, n, Ha, Wa] (raw or
        pooled); nv_int: dst interior sliceable by image. The first chunk is
        halved so the next layer's first matmul unblocks sooner."""
        ich = min(n, max(1, 1024 // (Ha * Wa)))
        bounds = [0]
        if ich >= 2:
            bounds.append(ich // 2)
        bounds += list(range(ich, n, ich)) + [n]
        for i0, i1 in zip(bounds, bounds[1:]):
            nc.scalar.activation(
                nv_int[:, i0:i1], srcv[:, i0:i1], AF.Relu, bias=bi[:], scale=sc[:]
            )

    def memset_borders(xv_full, Hn, Wn):
        nc.gpsimd.memset(xv_full[:, :, 0 : Hn : Hn - 1, :], 0.0)
        nc.gpsimd.memset(xv_full[:, :, 1 : Hn - 1, 0 : Wn : Wn - 1], 0.0)

    # ---------------- layer 1: row-tiled K=32 im2col conv ----------------
    xcv = At[:, 0:9248].rearrange("p (i h w) -> p i h w", h=34, w=34)
    y1 = Bt[:, B_Y[1] : B_Y[1] + 32768]
    y1v = y1.rearrange("p (i hf q) -> p i hf q", hf=2, q=512)
    st6l1 = misc.tile([128, 64 * 6], f32, tag="st6l1", name="st6_l1")
    st6v1 = st6l1[:].rearrange("p (t s) -> p t s", s=6)
    for s in range(16):
        i, hf = s // 2, s % 2
        hh = hf * 16
        accs = []
        for k in range(4):
            a = acc_tile(f"l1a{s}_{k}")
            nc.tensor.matmul(
                a[:], w1t[32 * k : 32 * k + 32, :],
                xcv[32 * k : 32 * k + 32, i, hh + 1 : hh + 17, 1:33],
                start=True, stop=True, tile_position=(32 * k, 0),
            )
            accs.append(a)
        for k in range(4):
            # stats straight from psum (vector) so the collective isn't gated
            # on the scalar casts; casts may lag into the collective window.
            nc.vector.bn_stats(st6v1[:, s * 4 + k], accs[k][:])
            nc.scalar.copy(y1v[:, 8 * k + i, hf], accs[k][:])

    x2v = At[:, 0:36992].rearrange("p (i h w) -> p i h w", h=34, w=34)
    memset_borders(x2v, 34, 34)

    gl1 = stats_and_collective(1, 0, st6v1)
    sc1, bi1 = finalize(1, 0, gl1)
    emit_apply(
        y1.rearrange("p (i h w) -> p i h w", h=32, w=32),
        x2v[:, :, 1:33, 1:33], 32, 32, sc1, bi1,
    )
    if "y1" in dbg:
        nc.sync.dma_start(out=dbg["y1"][:], in_=y1)

    # ---------------- conv layers 2..6 ----------------
    def conv_layer(l):
        ci, co, H, W, do_pool = CONV_CFG[l - 1]
        ci_t, co_t = max(1, ci // 128), co // 128
        npix = n * H * W
        ntile = npix // 512
        ipt = 512 // (H * W) if H * W <= 512 else 0
        Ho, Wo = (H // 2, W // 2) if do_pool else (H, W)
        npo = n * Ho * Wo

        if do_pool:
            yp = Bt[:, B_YP[l] : B_YP[l] + co_t * npo]
            yr = None
        else:
            yr = Bt[:, B_Y[l] : B_Y[l] + co_t * npix]
            yp = None

        if l == 2:
            srcv = x2v
        else:
            o0 = OFF_X[l]
            srcv = At[:, o0 : o0 + SZ_X[l]].rearrange(
                "p (t i h w) -> p t i h w", t=ci_t, h=H + 2, w=W + 2
            )
        o1 = OFF_X[l + 1]
        if l < 6:
            Hn, Wn = Ho + 2, Wo + 2
            nxtv = At[:, o1 : o1 + SZ_X[l + 1]].rearrange(
                "p (t i h w) -> p t i h w", t=co_t, h=Hn, w=Wn
            )
        else:
            nxtv = At[:, o1 : o1 + SZ_X[7]].rearrange(
                "p (t i q) -> p t i q", t=co_t, q=16
            )

        # padded-output borders: for l>=3 the dst region only overlaps data
        # dead by this layer's start, so zero it up front (off critical path);
        # l2's dst lives inside x2 (this layer's source) - done after tiles.
        if l in (3, 4, 5):
            nxtf = At[:, o1 : o1 + SZ_X[l + 1]].rearrange(
                "p (a h w) -> p a h w", h=Ho + 2, w=Wo + 2
            )
            memset_borders(nxtf, Ho + 2, Wo + 2)

        if l == 2:
            wv = w2t[:].rearrange("p (o c) -> p o c", o=9)
            wget = lambda t, o, ct: wv[:, o, :]
        elif l == 3:
            wv = w3t[:].rearrange("p (o c) -> p o c", o=9)
            wget = lambda t, o, ct: wv[:, o, ct * 128 : (ct + 1) * 128]
        elif l == 4:
            wv = w4t[:].rearrange("p (t o c) -> p t o c", t=2, o=9)
            wget = lambda t, o, ct: wv[:, t, o, ct * 128 : (ct + 1) * 128]
        else:
            wget = None

        for ct in range(co_t):
            if l in (5, 6):
                wc = Wp.tile(
                    [128, ci_t * 9 * 128], f16, tag=f"w{l}s", bufs=2, name=f"w{l}c{ct}"
                )
                nc.sync.dma_start(out=wc[:], in_=ins[f"w{l}s"][ct])
                wcv = wc[:].rearrange("p (t o c) -> p t o c", t=ci_t, o=9)
                wget = lambda t, o, _ct, wcv=wcv: wcv[:, t, o, :]
            st6 = misc.tile([128, 64 * 6], f32, tag="st6", bufs=2, name=f"st6_{l}_{ct}")
            st6v = st6[:, 0 : ntile * 6].rearrange("p (t s) -> p t s", s=6)
            for pt in range(ntile):
                a = acc_tile(f"a{l}_{ct}_{pt}")
                first = True
                for t in range(ci_t):
                    for dh in range(3):
                        for dw in range(3):
                            o = dh * 3 + dw
                            if ipt == 0:
                                img, hh = pt // 2, (pt % 2) * 16
                                rhs = srcv[:, img, hh + dh : hh + dh + 16, dw : dw + 32]
                            else:
                                i0 = pt * ipt
                                rhs = srcv[:, t, i0 : i0 + ipt, dh : dh + H, dw : dw + W]
                            nc.tensor.matmul(
                                a[:], wget(t, o, ct), rhs,
                                start=first, stop=(t == ci_t - 1 and o == 8),
                            )
                            first = False
                if not do_pool:
                    ydst = yr[:, ct * npix + pt * 512 : ct * npix + (pt + 1) * 512]
                    nc.scalar.copy(ydst, a[:])
                    nc.vector.bn_stats(st6v[:, pt], ydst)
                else:
                    nc.vector.bn_stats(st6v[:, pt], a[:])
                if do_pool:
                    # 2x2 max-pool straight from psum (commutes with the later
                    # monotone scale>0 BN+relu)
                    if ipt == 0:  # l2: half-image tile, 16x32 px
                        img, hf = pt // 2, pt % 2
                        av = a[:].rearrange(
                            "p (hp qh wp qw) -> p hp wp qh qw", hp=8, qh=2, qw=2
                        )
                        ypv2 = yp.rearrange("p (i h w) -> p i h w", h=16, w=16)
                        nc.vector.tensor_reduce(
                            ypv2[:, img, hf * 8 : hf * 8 + 8, :], av, AX.XY, OP.max
                        )
                    else:
                        av = a[:].rearrange(
                            "p (i hp qh wp qw) -> p i hp wp qh qw",
                            i=ipt, hp=H // 2, qh=2, qw=2,
                        )
                        ypv = yp.rearrange(
                            "p (t i h w) -> p t i h w", t=co_t, h=Ho, w=Wo
                        )
                        i0 = pt * ipt
                        nc.vector.tensor_reduce(
                            ypv[:, ct, i0 : i0 + ipt], av, AX.XY, OP.max
                        )
                else:
                    pass  # handled above (scalar copy + f16 stats)

            if l == 2:  # x3 borders: right after l2's matmuls, before apply
                nxtf = At[:, o1 : o1 + SZ_X[3]].rearrange(
                    "p (a h w) -> p a h w", h=Ho + 2, w=Wo + 2
                )
                memset_borders(nxtf, Ho + 2, Wo + 2)

            gl = stats_and_collective(l, ct, st6v)
            sc, bi = finalize(l, ct, gl)
            if l < 6:
                dst = nxtv[:, ct, :, 1 : Ho + 1, 1 : Wo + 1]
            else:
                dst = nxtv[:, ct]
            if do_pool:
                src_ap = yp.rearrange(
                    "p (t i h w) -> p t i h w", t=co_t, h=Ho, w=Wo
                )[:, ct]
            else:
                src_ap = yr[:, ct * npix : (ct + 1) * npix].rearrange(
                    "p (i h w) -> p i h w", h=H, w=W
                )
            emit_apply(src_ap, dst, Ho, Wo, sc, bi)

        # fc-weight pre-staging in dead arena space
        if l == 4:
            for m in range(N_PRE_A, N_PRE_A + N_PRE_B):
                o = OFF_FW1B + (m - N_PRE_A) * 1024
                nc.sync.dma_start(out=Bt[:, o : o + 1024], in_=ins["fw1s"][m])
            for m in range(N_PRE_A + N_PRE_B, PRE_TOT):
                o = OFF_FW1C + (m - N_PRE_A - N_PRE_B) * 1024
                nc.sync.dma_start(out=Bt[:, o : o + 1024], in_=ins["fw1s"][m])
        if l == 5:
            for m in range(N_PRE_A):
                o = OFF_FW1A + m * 1024
                nc.sync.dma_start(out=At[:, o : o + 1024], in_=ins["fw1s"][m])
        if l == 6:
            for jt in range(8):
                o = OFF_FW2 + jt * 1024
                nc.sync.dma_start(
                    out=Bt[:, o : o + 1024],
                    in_=ins["fw2t"][jt * 128 : (jt + 1) * 128, :],
                )

        if f"y{l}" in dbg:
            src = yp if do_pool else yr
            sz = npo if do_pool else npix
            for ct in range(co_t):
                nc.sync.dma_start(
                    out=dbg[f"y{l}"][ct * 128 : (ct + 1) * 128, :],
                    in_=src[:, ct * sz : (ct + 1) * sz],
                )

    for l in range(2, 7):
        conv_layer(l)

    # ---------------- FC ----------------
    xfcv = At[:, 0:2048].rearrange("p (t i q) -> p t i q", t=4, q=16)
    if "xfc" in dbg:
        for t in range(4):
            nc.sync.dma_start(
                out=dbg["xfc"][t * 128 : (t + 1) * 128, :], in_=xfcv[:, t]
            )

    fb1b = misc.tile([1, 1024], f16, tag="fb1b")
    nc.sync.dma_start(out=fb1b[:], in_=ins["fb1"][:])
    fb2b = misc.tile([1, 1024], f16, tag="fb2b")
    nc.sync.dma_start(out=fb2b[:], in_=ins["fb2"][:])
    fb3f = misc.tile([1, 10], f32, tag="fb3f")
    nc.sync.dma_start(out=fb3f[:], in_=ins["fb3"][:])
    ones_b = misc.tile([1, n], f16, tag="ones_b")
    nc.vector.memset(ones_b[:], 1.0)
    ones_f = misc.tile([1, n], f32, tag="ones_f")
    nc.vector.memset(ones_f[:], 1.0)
    idb = misc.tile([n, n], f16, tag="idb")
    make_identity(nc, idb[:])
    idf = misc.tile([n, n], f32, tag="idf")
    make_identity(nc, idf[:])

    acc_h = [acc_tile(f"fc1acc{h}") for h in range(2)]
    fw1pa = At[:, OFF_FW1A : OFF_FW1A + N_PRE_A * 1024].rearrange(
        "p (m q) -> p m q", q=1024
    )
    for m in range(64):
        ct, p = divmod(m, 16)
        if m < N_PRE_A:
            wch = fw1pa[:, m]
        elif m < N_PRE_A + N_PRE_B:
            o = OFF_FW1B + (m - N_PRE_A) * 1024
            wch = Bt[:, o : o + 1024]
        elif m < PRE_TOT:
            o = OFF_FW1C + (m - N_PRE_A - N_PRE_B) * 1024
            wch = Bt[:, o : o + 1024]
        else:
            slot = (m - PRE_TOT) % 14
            if slot < 8:
                wch = Bt[:, OFF_FC1WIN + slot * 1024 : OFF_FC1WIN + (slot + 1) * 1024]
            elif slot < 12:  # dead x5 space in arena A
                o = 2048 + (slot - 8) * 1024
                wch = At[:, o : o + 1024]
            else:  # dead yp4 space in arena B
                o = 2048 + (slot - 12) * 1024
                wch = Bt[:, o : o + 1024]
            nc.sync.dma_start(out=wch, in_=ins["fw1s"][m])
        lhsT = xfcv[:, ct, :, p]
        for hh in range(2):
            nc.tensor.matmul(
                acc_h[hh][0:32, :], lhsT, wch[:, hh * 512 : (hh + 1) * 512],
                start=(m == 0), stop=False,
            )
    y1fc = misc.tile([n, 1024], f16, tag="y1fc")
    for hh in range(2):
        nc.tensor.matmul(
            acc_h[hh][0:32, :], ones_b[:], fb1b[:, hh * 512 : (hh + 1) * 512],
            start=False, stop=True,
        )
        nc.scalar.activation(
            y1fc[:, hh * 512 : (hh + 1) * 512], acc_h[hh][0:32, :], AF.Relu
        )
    if "yfc1" in dbg:
        nc.sync.dma_start(out=dbg["yfc1"][:], in_=y1fc[:])

    y1t = misc.tile([128, 8 * n], f16, tag="y1t")
    y1tv = y1t[:].rearrange("p (t i) -> p t i", t=8)
    tps = []
    for jt in range(8):
        tp = acc_tile(f"tr1_{jt}")
        tpb = tp[:].bitcast(f16)[:, 0:n]
        nc.tensor.transpose(tpb, y1fc[:, jt * 128 : (jt + 1) * 128], idb[:])
        tps.append(tpb)
    for jt in range(8):
        nc.vector.tensor_copy(y1tv[:, jt], tps[jt])

    w2fv = Bt[:, OFF_FW2 : OFF_FW2 + 8 * 1024].rearrange("p (t q) -> p t q", q=1024)
    y2fc = misc.tile([n, 1024], f32, tag="y2fc")
    for hh in range(2):
        a2 = acc_tile(f"fc2acc{hh}")
        for jt in range(8):
            nc.tensor.matmul(
                a2[0:32, :], y1tv[:, jt], w2fv[:, jt, hh * 512 : (hh + 1) * 512],
                start=(jt == 0), stop=False,
            )
        nc.tensor.matmul(
            a2[0:32, :], ones_b[:], fb2b[:, hh * 512 : (hh + 1) * 512],
            start=False, stop=True,
        )
        nc.scalar.activation(
            y2fc[:, hh * 512 : (hh + 1) * 512], a2[0:32, :], AF.Relu
        )
    if "yfc2" in dbg:
        nc.sync.dma_start(out=dbg["yfc2"][:], in_=y2fc[:])

    y2t = misc.tile([128, 8 * n], f32, tag="y2t")
    y2tv = y2t[:].rearrange("p (t i) -> p t i", t=8)
    tps2 = []
    for it in range(8):
        tp = acc_tile(f"tr2_{it}")
        tpf = tp[:][:, 0:n]
        nc.tensor.transpose(tpf, y2fc[:, it * 128 : (it + 1) * 128], idf[:])
        tps2.append(tpf)
    for it in range(8):
        nc.vector.tensor_copy(y2tv[:, it], tps2[it])
    w3fc = misc.tile([128, 8 * 10], f32, tag="w3fc")
    w3v = w3fc[:].rearrange("p (t j) -> p t j", j=10)
    nc.sync.dma_start(out=w3v, in_=ins["fw3t"][:].rearrange("(t c) j -> c t j", c=128))
    a3 = acc_tile("fc3acc")
    for it in range(8):
        nc.tensor.matmul(
            a3[0:n, 0:10], y2tv[:, it], w3v[:, it], start=(it == 0), stop=False
        )
    nc.tensor.matmul(a3[0:n, 0:10], ones_f[:], fb3f[:], start=False, stop=True)
    out_sb = misc.tile([n, 10], f32, tag="out_sb")
    nc.scalar.copy(out_sb[:], a3[0:n, 0:10])
    nc.sync.dma_start(out=ins["out"][:], in_=out_sb[:])

    for p in (misc, Wp, Bp, Ap, psum):
        p.release()


# ---------------------------------------------------------------------------
# host-side wrapper (layout/transpose/binarize only)
# ---------------------------------------------------------------------------

_CACHE = {}


def _binarize(a):
    return np.where(np.asarray(a, np.float32) >= 0, 1.0, -1.0).astype(np.float32)


def _prep_inputs(inputs):
    h = np.float16
    sh = {}
    w1b = _binarize(inputs["cw1"])  # [128, 3, 3, 3] OIHW
    w1c = w1b.transpose(2, 3, 1, 0).reshape(27, 128)
    w1col = np.zeros((128, 128), np.float32)
    for k in range(4):
        w1col[32 * k : 32 * k + 27] = w1c
    sh["w1"] = w1col.astype(h)

    def conv_w(l):
        cw = _binarize(inputs[f"cw{l}"])  # [co, ci, 3, 3]
        co, ci = cw.shape[0], cw.shape[1]
        arr = cw.transpose(2, 3, 1, 0).reshape(9, ci, co)  # [o, ci, co]
        t = ci // 128
        a = arr.transpose(1, 0, 2).reshape(t, 128, 9, co).transpose(1, 0, 2, 3)
        return np.ascontiguousarray(a.reshape(128, t * 9 * co)).astype(h)

    sh["w2"], sh["w3"], sh["w4"] = conv_w(2), conv_w(3), conv_w(4)

    def conv_w_ct(l):
        cw = _binarize(inputs[f"cw{l}"])
        co, ci = cw.shape[0], cw.shape[1]
        arr = cw.transpose(2, 3, 1, 0).reshape(9, ci, co)
        t, nct = ci // 128, co // 128
        outw = np.zeros((nct, 128, t * 9 * 128), np.float32)
        for c in range(nct):
            ch = arr[:, :, c * 128 : (c + 1) * 128]
            a = ch.transpose(1, 0, 2).reshape(t, 128, 9, 128).transpose(1, 0, 2, 3)
            outw[c] = a.reshape(128, t * 9 * 128)
        return outw.astype(h)

    sh["w5s"], sh["w6s"] = conv_w_ct(5), conv_w_ct(6)
    for l in range(1, 7):
        sh[f"g{l}"] = np.ascontiguousarray(inputs[f"g{l}"], np.float32)
        sh[f"bt{l}"] = np.ascontiguousarray(inputs[f"bt{l}"], np.float32)
    fw1 = _binarize(inputs["fw1"])  # [1024, 8192]
    a = fw1.reshape(1024, 512, 16).transpose(1, 2, 0)  # [ch, px, out]
    a = a.reshape(4, 128, 16, 1024).transpose(0, 2, 1, 3)  # [ct, px, part, out]
    sh["fw1s"] = np.ascontiguousarray(a.reshape(64, 128, 1024)).astype(h)
    sh["fw2t"] = np.ascontiguousarray(_binarize(inputs["fw2"]).T).astype(h)
    sh["fw3t"] = np.ascontiguousarray(np.asarray(inputs["fw3"], np.float32).T)
    sh["fb1"] = np.asarray(inputs["fb1"], np.float32).reshape(1, 1024).astype(h)
    sh["fb2"] = np.asarray(inputs["fb2"], np.float32).reshape(1, 1024).astype(h)
    sh["fb3"] = np.ascontiguousarray(
        np.asarray(inputs["fb3"], np.float32).reshape(1, 10)
    )

    x = np.asarray(inputs["x"], np.float32)
    xp = np.zeros((256, 3, 34, 34), np.float32)
    xp[:, :, 1:33, 1:33] = x
    xpf = xp.reshape(256, 3, 1156)
    taps = [(dh, dw) for dh in range(3) for dw in range(3)]
    in_maps = []
    for c in range(N_CORES):
        xc = np.zeros((128, 8, 1156), np.float32)
        for k in range(4):
            imgs = xpf[c * 32 + 8 * k : c * 32 + 8 * k + 8]
            for o, (dh, dw) in enumerate(taps):
                s = (dh - 1) * 34 + (dw - 1)
                d0, d1 = max(0, -s), 1156 - max(0, s)
                xc[32 * k + o * 3 : 32 * k + o * 3 + 3, :, d0:d1] = imgs[
                    :, :, d0 + s : d1 + s
                ].transpose(1, 0, 2)
        m = dict(sh)
        m["xcol"] = xc.reshape(128, 8 * 1156).astype(h)
        in_maps.append(m)
    return in_maps


def run(inputs, debug=False, trace=False):
    key = "dbg" if debug else "rel"
    if key not in _CACHE:
        _CACHE[key] = build(debug=debug)
    nc = _CACHE[key]
    in_maps = _prep_inputs(inputs)
    res = run_bass_kernel_spmd(nc, in_maps, core_ids=list(range(N_CORES)), trace=trace)
    outs = np.concatenate([r["out"] for r in res.results], axis=0)
    return outs, res


def kernel(**inputs) -> np.ndarray:
    outs, _ = run(inputs, debug=False, trace=False)
    return outs
